# revision 7
# baseline (speedup 1.0000x reference)
"""Trainium2 Bass kernel for the complex AttnBlock (GroupNorm + complex 1x1-conv
attention) — data-parallel over batch B=8 across 8 NeuronCores.

Math notes (per sample):
  x = xr + i*xi, h = GN(xr) + i*GN(xi)           [C=256, HW=1024]
  q/k/v complex 1x1 convs; attention logits only need
  Re(<q, conj(k)>): S[n,m] = sum_c qr[c,n]kr[c,m] + qi[c,n]ki[c,m]
  A = softmax(S.real) is REAL, so hf = A @ v acts on re/im independently.
  Everything is computed in a transpose-free layout:
    St[m,n] = k^T q         (lhsT = k, rhs = q, both natural [c, *])
    v^T[m,o] = h^T Wv^T     (lhsT = h, rhs = WvT, both natural)
    hh[c,n] = v^T.T @ expSt (lhsT = v^T, rhs = expSt, both natural)
  Softmax: logits are bounded (~|8|) so exp without max-subtraction is safe;
  1/sqrt(C) is folded into Wq host-side; the 1/colsum normalization is folded
  into the PSUM->SBUF evacuation of hh (one tensor_mul with a broadcast tile
  built by a K=1 outer-product matmul; 1/colsum via reciprocal_approx_fast to
  avoid ACT table switches).
  wo ~ 1e-5 means the attention branch contributes ~1e-5 of the output
  (out = x + tiny), so bf16 matmuls (1 cyc/row on PE vs 4 for fp32) are safe.

Scheduling notes:
  - input DMAs ordered x -> emat/ebmat -> wall (x gates the GroupNorm chain)
  - WARM_MM dummy matmuls on a scratch tile warm the PE HAM clock-gate during
    the DMA + GroupNorm head so real matmuls run at 2.4 GHz from the start
  - GroupNorm is emitted phase-wise across the 4 (part, chunk) tiles so the
    DVE FIFO never stalls behind another tile's cross-engine round-trip
  - PSUM->SBUF evacuations alternate between ScalarE and VectorE
  - the frep outer-product matmuls are emitted after the first hh group so
    the PE FIFO doesn't stall waiting for the reciprocal chain
"""

import sys

sys.path.insert(0, "/opt/trn_rl_repo")

import numpy as np
import ml_dtypes

import concourse.bacc as bacc
import concourse.tile as tile
from concourse import mybir
from concourse.bass_utils import run_bass_kernel_spmd

F32 = mybir.dt.float32
BF16 = mybir.dt.bfloat16
AF = mybir.ActivationFunctionType
OP = mybir.AluOpType

B, C, H, W = 8, 256, 32, 32
HW = H * W
G = 32
EPS = 1e-5
NCORES = 8
CK = C // 128      # channel chunks (2)
NK = HW // 512     # free-dim n chunks of 512 (2)
MK = HW // 128     # hw chunks of 128 (8)
GPC = 16           # groups per channel-chunk
WARM_MM = 32       # HAM warm-up matmuls at kernel start


def _widx(proj, kind, ci):
    # proj: 0=q 1=k 2=v 3=o ; kind: 0=re 1=im 2=neg-im ; ci: channel chunk
    return proj * 6 + kind * 2 + ci


def _build_nc(affine_trivial: bool, bias_zero: bool):
    nc = bacc.Bacc("TRN2", target_bir_lowering=False, debug=False)

    x_d = nc.dram_tensor("x", [2, C, HW], F32, kind="ExternalInput")
    w_d = nc.dram_tensor("wall", [128, 24, 256], BF16, kind="ExternalInput")
    e_d = nc.dram_tensor("emat", [128, GPC], F32, kind="ExternalInput")
    eb_d = nc.dram_tensor("ebmat", [GPC, 128], F32, kind="ExternalInput")
    gn_d = None
    if not affine_trivial:
        gn_d = nc.dram_tensor("gnwb", [2, 2, C], F32, kind="ExternalInput")
    bias_d = None
    if not bias_zero:
        bias_d = nc.dram_tensor("bias", [4, 2, C], BF16, kind="ExternalInput")
    out_d = nc.dram_tensor("out", [2, C, HW], F32, kind="ExternalOutput")

    with tile.TileContext(nc) as tc:
        with (
            tc.tile_pool(name="const", bufs=1) as constp,
            tc.tile_pool(name="xp", bufs=1) as xp,
            tc.tile_pool(name="hp", bufs=1) as hp,
            tc.tile_pool(name="qkp", bufs=1) as qkp,
            tc.tile_pool(name="vtp", bufs=1) as vtp,
            tc.tile_pool(name="estp", bufs=1) as estp,
            tc.tile_pool(name="hhp", bufs=1) as hhp,
            tc.tile_pool(name="frp", bufs=1) as frp,
            tc.tile_pool(name="outp", bufs=1) as outp,
            tc.tile_pool(name="smallp", bufs=4) as smallp,
            tc.tile_pool(name="psp", bufs=7, space="PSUM") as psp,
            tc.tile_pool(name="psw", bufs=1, space="PSUM") as psw,
        ):
            # ---- input DMAs (x first: it gates the GroupNorm chain) ----
            xs = [[None] * CK for _ in range(2)]
            for part in range(2):
                for ci in range(CK):
                    xt = xp.tile([128, HW], F32, tag=f"x{part}{ci}")
                    nc.sync.dma_start(xt[:], x_d[part, ci * 128:(ci + 1) * 128, :])
                    xs[part][ci] = xt
            emat = constp.tile([128, GPC], F32, tag="emat")
            nc.sync.dma_start(emat[:], e_d[:])
            ebmat = constp.tile([GPC, 128], F32, tag="ebmat")
            nc.sync.dma_start(ebmat[:], eb_d[:])
            wall = constp.tile([128, 24, 256], BF16, tag="wall")
            nc.sync.dma_start(wall[:], w_d[:])

            eps16 = constp.tile([GPC, 1], F32, tag="eps16")
            nc.vector.memset(eps16[:], EPS)
            onescol = constp.tile([128, 1], BF16, tag="onescol")
            nc.vector.memset(onescol[:], 1.0)
            onesrow = constp.tile([1, 128], F32, tag="onesrow")
            nc.vector.memset(onesrow[:], 1.0)
            warm_sb = constp.tile([128, 512], BF16, tag="warm_sb")
            nc.vector.memset(warm_sb[:], 0.0)
            if not bias_zero:
                onesrow_bf = constp.tile([1, 128], BF16, tag="onesrow_bf")
                nc.vector.memset(onesrow_bf[:], 1.0)
                ones_n = constp.tile([1, 512], BF16, tag="ones_n")
                nc.vector.memset(ones_n[:], 1.0)
                bias_t = [[None, None] for _ in range(4)]
                for pj in range(4):
                    for part in range(2):
                        bt = constp.tile([1, C], BF16, tag=f"bias{pj}{part}")
                        nc.sync.dma_start(
                            bt[:], bias_d[pj, part, :].rearrange("(o c) -> o c", o=1))
                        bias_t[pj][part] = bt
            if not affine_trivial:
                gwb_t = [[None, None] for _ in range(2)]  # [wb][part] -> [128, CK]
                for wb in range(2):
                    for part in range(2):
                        gt = constp.tile([128, CK], F32, tag=f"gn{wb}{part}")
                        nc.sync.dma_start(
                            gt[:], gn_d[wb, part, :].rearrange("(ci p) -> p ci", p=128))
                        gwb_t[wb][part] = gt

            # ---- PE HAM warm-up: dummy matmuls on scratch data ----
            pwarm = psw.tile([128, 512], F32, tag="warm")
            for _ in range(WARM_MM):
                nc.tensor.matmul(pwarm[:], warm_sb[:, 0:128], warm_sb[:],
                                 start=True, stop=True)

            # ---- GroupNorm -> h (bf16), phase-wise across the 4 tiles ----
            tiles4 = [(part, ci) for part in range(2) for ci in range(CK)]
            st6s, mvs, psgs, gcps, gags, gsbs, psbs, scvs = ({} for _ in range(8))
            for part, ci in tiles4:
                xt = xs[part][ci]
                st6 = smallp.tile([128, 2, 6], F32)
                nc.vector.bn_stats(st6[:, 0, :], xt[:, 0:512])
                nc.vector.bn_stats(st6[:, 1, :], xt[:, 512:1024])
                st6s[part, ci] = st6
            for part, ci in tiles4:
                mv = smallp.tile([128, 3], F32)
                nc.vector.bn_aggr(mv[:, 0:2], st6s[part, ci][:])
                nc.vector.tensor_mul(mv[:, 2:3], mv[:, 0:1], mv[:, 0:1])
                mvs[part, ci] = mv
            for part, ci in tiles4:
                # group stats [mean_g, E[var]_g, E[mean^2]_g] (emat = 1/8 indicator)
                psg = psp.tile([GPC, 3], F32, tag="ps")
                nc.tensor.matmul(psg[:], emat[:], mvs[part, ci][:],
                                 start=True, stop=True)
                psgs[part, ci] = psg
            for part, ci in tiles4:
                gcp = smallp.tile([GPC, 3], F32)
                nc.vector.tensor_copy(gcp[:], psgs[part, ci][:])
                gcps[part, ci] = gcp
            for part, ci in tiles4:
                gcp = gcps[part, ci]
                gag = smallp.tile([GPC, 4], F32)
                nc.vector.tensor_add(gag[:, 0:1], gcp[:, 1:2], gcp[:, 2:3])
                nc.vector.tensor_mul(gag[:, 1:2], gcp[:, 0:1], gcp[:, 0:1])
                nc.vector.tensor_sub(gag[:, 2:3], gag[:, 0:1], gag[:, 1:2])
                nc.scalar.activation(gag[:, 3:4], gag[:, 2:3], AF.Sqrt, bias=eps16[:])
                gags[part, ci] = gag
            for part, ci in tiles4:
                gsb = smallp.tile([GPC, 2], F32)
                nc.vector.reciprocal(gsb[:, 1:2], gags[part, ci][:, 3:4])
                nc.vector.tensor_copy(gsb[:, 0:1], gcps[part, ci][:, 0:1])
                gsbs[part, ci] = gsb
            for part, ci in tiles4:
                psb = psp.tile([128, 2], F32, tag="ps")
                nc.tensor.matmul(psb[:], ebmat[:], gsbs[part, ci][:],
                                 start=True, stop=True)
                psbs[part, ci] = psb
            hs = [[None] * CK for _ in range(2)]
            for part, ci in tiles4:
                scv = smallp.tile([128, 2], F32)
                nc.vector.tensor_copy(scv[:], psbs[part, ci][:])
                ht = hp.tile([128, HW], BF16, tag=f"h{part}{ci}")
                if affine_trivial:
                    nc.vector.tensor_scalar(
                        out=ht[:], in0=xs[part][ci][:],
                        scalar1=scv[:, 0:1], scalar2=scv[:, 1:2],
                        op0=OP.subtract, op1=OP.mult)
                else:
                    ab = smallp.tile([128, 2], F32)
                    # A = rstd * gn_w ; B = gn_b - mean * A
                    nc.vector.tensor_mul(
                        ab[:, 0:1], scv[:, 1:2], gwb_t[0][part][:, ci:ci + 1])
                    nc.vector.tensor_mul(ab[:, 1:2], scv[:, 0:1], ab[:, 0:1])
                    nc.vector.tensor_sub(
                        ab[:, 1:2], gwb_t[1][part][:, ci:ci + 1], ab[:, 1:2])
                    nc.vector.tensor_scalar(
                        out=ht[:], in0=xs[part][ci][:],
                        scalar1=ab[:, 0:1], scalar2=ab[:, 1:2],
                        op0=OP.mult, op1=OP.add)
                hs[part][ci] = ht

            # ---- q, k projections (out layout [o, n]) ----
            # q_re = Wqr h_re - Wqi h_im ; q_im = Wqr h_im + Wqi h_re
            qk = [[[None] * CK for _ in range(2)] for _ in range(2)]  # [pj][part][mo]
            evac_flip = 0
            for pj in range(2):  # 0=q 1=k
                for part in range(2):
                    terms = [(0, hs[0]), (2, hs[1])] if part == 0 else \
                            [(0, hs[1]), (1, hs[0])]
                    for mo in range(CK):
                        qt = qkp.tile([128, HW], BF16, tag=f"qk{pj}{part}{mo}",
                                      name=f"qk{pj}{part}{mo}")
                        qk[pj][part][mo] = qt
                        for nn in range(NK):
                            ps = psp.tile([128, 512], F32, tag="ps")
                            nmm = 4 if bias_zero else 5
                            mm = 0
                            for kind, hsrc in terms:
                                for ci in range(CK):
                                    nc.tensor.matmul(
                                        ps[:],
                                        wall[:, _widx(pj, kind, ci),
                                             mo * 128:(mo + 1) * 128],
                                        hsrc[ci][:, nn * 512:(nn + 1) * 512],
                                        start=(mm == 0), stop=(mm == nmm - 1))
                                    mm += 1
                            if not bias_zero:
                                nc.tensor.matmul(
                                    ps[:],
                                    bias_t[pj][part][:, mo * 128:(mo + 1) * 128],
                                    ones_n[:], start=False, stop=True)
                            dst = qt[:, nn * 512:(nn + 1) * 512]
                            if evac_flip % 2 == 0:
                                nc.scalar.copy(dst, ps[:])
                            else:
                                nc.vector.tensor_copy(dst, ps[:])
                            evac_flip += 1

            # ---- v^T (layout [m, o]) ----
            vt = [None, None]
            for part in range(2):
                vt[part] = vtp.tile([128, MK, 256], BF16, tag=f"vt{part}",
                                    name=f"vt{part}")
            for part in range(2):
                terms = [(hs[0], 0), (hs[1], 2)] if part == 0 else \
                        [(hs[1], 0), (hs[0], 1)]
                for mk in range(MK):
                    ps = psp.tile([128, 256], F32, tag="ps")
                    nmm = 4 if bias_zero else 5
                    mm = 0
                    for hsrc, kind in terms:
                        for ci in range(CK):
                            nc.tensor.matmul(
                                ps[:],
                                hsrc[ci][:, mk * 128:(mk + 1) * 128],
                                wall[:, _widx(2, kind, ci), :],
                                start=(mm == 0), stop=(mm == nmm - 1))
                            mm += 1
                    if not bias_zero:
                        nc.tensor.matmul(ps[:], onesrow_bf[:], bias_t[2][part][:],
                                         start=False, stop=True)
                    if evac_flip % 2 == 0:
                        nc.scalar.copy(vt[part][:, mk, :], ps[:])
                    else:
                        nc.vector.tensor_copy(vt[part][:, mk, :], ps[:])
                    evac_flip += 1

            # ---- St = k^T q (1/16 folded into wq) -> exp (bf16) ----
            est = [None] * MK
            for mk in range(MK):
                est[mk] = estp.tile([128, HW], BF16, tag=f"est{mk}", name=f"est{mk}")
            for mk in range(MK):
                for nn in range(NK):
                    ps = psp.tile([128, 512], F32, tag="ps")
                    mm = 0
                    for part in range(2):
                        for ci in range(CK):
                            nc.tensor.matmul(
                                ps[:],
                                qk[1][part][ci][:, mk * 128:(mk + 1) * 128],
                                qk[0][part][ci][:, nn * 512:(nn + 1) * 512],
                                start=(mm == 0), stop=(mm == 3))
                            mm += 1
                    nc.scalar.activation(
                        est[mk][:, nn * 512:(nn + 1) * 512], ps[:], AF.Exp)

            # ---- column sums -> 1/colsum (DVE approx reciprocal) ----
            ivcs = [None] * NK
            for nn in range(NK):
                psc = psp.tile([1, 512], F32, tag="ps")
                for mk in range(MK):
                    nc.tensor.matmul(
                        psc[:], onescol[:], est[mk][:, nn * 512:(nn + 1) * 512],
                        start=(mk == 0), stop=(mk == MK - 1))
                ivc = smallp.tile([1, 512], F32)
                nc.vector.reciprocal_approx_fast(out=ivc[:], in_=psc[:])
                ivcs[nn] = ivc

            # ---- hh = v^T.T @ expSt, normalized during evacuation ----
            # frep (ones outer ivc) is emitted after the first hh group so the
            # PE FIFO never waits on the reciprocal chain.
            hh = [[None] * CK for _ in range(2)]
            frep = [None] * NK
            for part in range(2):
                for co in range(CK):
                    hh[part][co] = hhp.tile([128, HW], BF16, tag=f"hh{part}{co}",
                                            name=f"hh{part}{co}")
            hh_groups = [(part, co, nn)
                         for part in range(2) for co in range(CK) for nn in range(NK)]
            for gi, (part, co, nn) in enumerate(hh_groups):
                ps = psp.tile([128, 512], F32, tag="ps")
                for mk in range(MK):
                    nc.tensor.matmul(
                        ps[:],
                        vt[part][:, mk, co * 128:(co + 1) * 128],
                        est[mk][:, nn * 512:(nn + 1) * 512],
                        start=(mk == 0), stop=(mk == MK - 1))
                if gi == 0:
                    for fn in range(NK):
                        psf = psp.tile([128, 512], F32, tag="ps")
                        nc.tensor.matmul(psf[:], onesrow[:], ivcs[fn][:],
                                         start=True, stop=True)
                        ft = frp.tile([128, 512], F32, tag=f"frep{fn}",
                                      name=f"frep{fn}")
                        nc.vector.tensor_copy(ft[:], psf[:])
                        frep[fn] = ft
                nc.vector.tensor_mul(
                    hh[part][co][:, nn * 512:(nn + 1) * 512], ps[:], frep[nn][:])

            # ---- z = Wo hh (complex), out = x + z ----
            for part in range(2):
                terms = [(0, hh[0]), (2, hh[1])] if part == 0 else \
                        [(0, hh[1]), (1, hh[0])]
                for mo in range(CK):
                    ot = outp.tile([128, HW], F32, tag=f"out{part}{mo}",
                                   name=f"out{part}{mo}")
                    for nn in range(NK):
                        ps = psp.tile([128, 512], F32, tag="ps")
                        nmm = 4 if bias_zero else 5
                        mm = 0
                        for kind, hsrc in terms:
                            for ci in range(CK):
                                nc.tensor.matmul(
                                    ps[:],
                                    wall[:, _widx(3, kind, ci),
                                         mo * 128:(mo + 1) * 128],
                                    hsrc[ci][:, nn * 512:(nn + 1) * 512],
                                    start=(mm == 0), stop=(mm == nmm - 1))
                                mm += 1
                        if not bias_zero:
                            nc.tensor.matmul(
                                ps[:],
                                bias_t[3][part][:, mo * 128:(mo + 1) * 128],
                                ones_n[:], start=False, stop=True)
                        nc.vector.tensor_add(
                            ot[:, nn * 512:(nn + 1) * 512], ps[:],
                            xs[part][mo][:, nn * 512:(nn + 1) * 512])
                    nc.sync.dma_start(out_d[part, mo * 128:(mo + 1) * 128, :], ot[:])

    nc.compile()
    return nc


_NC_CACHE = {}


def _get_nc(affine_trivial, bias_zero):
    key = (affine_trivial, bias_zero)
    if key not in _NC_CACHE:
        _NC_CACHE[key] = _build_nc(affine_trivial, bias_zero)
    return _NC_CACHE[key]


def _host_inputs(x2, gn_w, gn_b, wq, bq, wk, bk, wv, bv, wo, bo):
    bf = ml_dtypes.bfloat16
    sq = 1.0 / np.sqrt(np.float32(C))
    blocks = []
    for w, s in ((wq, sq), (wk, 1.0), (wv, 1.0), (wo, 1.0)):
        wr = np.asarray(w[0], np.float32).T * s
        wi = np.asarray(w[1], np.float32).T * s
        for mat in (wr, wi, -wi):
            for ci in range(CK):
                blocks.append(mat[ci * 128:(ci + 1) * 128, :])
    wall = np.ascontiguousarray(
        np.stack(blocks).transpose(1, 0, 2)).astype(bf)  # [128, 24, 256]

    emat = np.zeros((128, GPC), np.float32)
    ebmat = np.zeros((GPC, 128), np.float32)
    for c in range(128):
        emat[c, c // 8] = 0.125
        ebmat[c // 8, c] = 1.0

    gn_w = np.asarray(gn_w, np.float32)
    gn_b = np.asarray(gn_b, np.float32)
    affine_trivial = bool(np.all(gn_w == 1.0) and np.all(gn_b == 0.0))
    biases = np.stack([np.asarray(b, np.float32) for b in (bq, bk, bv, bo)])
    bias_zero = bool(np.all(biases == 0.0))

    shared = {"wall": wall, "emat": emat, "ebmat": ebmat}
    if not affine_trivial:
        shared["gnwb"] = np.ascontiguousarray(np.stack([gn_w, gn_b]))
    if not bias_zero:
        shared["bias"] = np.ascontiguousarray(biases).astype(bf)

    x2 = np.asarray(x2, np.float32)
    in_maps = []
    for b in range(B):
        m = dict(shared)
        m["x"] = np.ascontiguousarray(x2[:, b].reshape(2, C, HW))
        in_maps.append(m)
    return in_maps, affine_trivial, bias_zero


def kernel(x2, gn_w, gn_b, wq, bq, wk, bk, wv, bv, wo, bo, _profile_dir=None):
    in_maps, affine_trivial, bias_zero = _host_inputs(
        x2, gn_w, gn_b, wq, bq, wk, bk, wv, bv, wo, bo)
    nc = _get_nc(affine_trivial, bias_zero)

    if _profile_dir is not None:
        import ctypes, os
        import jax
        jax.devices()
        lib = ctypes.CDLL("/opt/axon/libaxon_pjrt.so")
        lib.axon_start_nrt_profile.argtypes = [
            ctypes.POINTER(ctypes.c_int64), ctypes.c_size_t]
        lib.axon_start_nrt_profile.restype = ctypes.c_int64
        lib.axon_stop_nrt_profile.argtypes = [ctypes.c_char_p]
        lib.axon_stop_nrt_profile.restype = ctypes.c_int64
        os.makedirs(_profile_dir, exist_ok=True)
        ids = (ctypes.c_int64 * NCORES)(*range(NCORES))
        rc = lib.axon_start_nrt_profile(ids, NCORES)
        if rc != 0:
            raise RuntimeError(f"axon_start_nrt_profile rc={rc}")
        try:
            res = run_bass_kernel_spmd(nc, in_maps, list(range(NCORES)))
        finally:
            n = lib.axon_stop_nrt_profile(_profile_dir.encode())
            print(f"profile: {n} file(s) written to {_profile_dir}")
    else:
        res = run_bass_kernel_spmd(nc, in_maps, list(range(NCORES)))

    out = np.stack(
        [np.asarray(res.results[b]["out"], np.float32) for b in range(B)], axis=1)
    return np.ascontiguousarray(out.reshape(2, B, C, H, W))


# revision 11
# speedup vs baseline: 1.0061x; 1.0061x over previous
"""Trainium2 Bass kernel for the complex AttnBlock (GroupNorm + complex 1x1-conv
attention) — data-parallel over batch B=8 across 8 NeuronCores.

Math notes (per sample):
  x = xr + i*xi, h = GN(xr) + i*GN(xi)           [C=256, HW=1024]
  q/k/v complex 1x1 convs; attention logits only need
  Re(<q, conj(k)>): S[n,m] = sum_c qr[c,n]kr[c,m] + qi[c,n]ki[c,m]
  A = softmax(S.real) is REAL, so hf = A @ v acts on re/im independently.
  Everything is computed in a transpose-free layout:
    St[m,n] = k^T q         (lhsT = k, rhs = q, both natural [c, *])
    v^T[m,o] = h^T Wv^T     (lhsT = h, rhs = WvT, both natural)
    hh[c,n] = v^T.T @ expSt (lhsT = v^T, rhs = expSt, both natural)
  Softmax: logits are bounded (~|8|) so exp without max-subtraction is safe;
  1/sqrt(C) is folded into Wq host-side; the 1/colsum normalization is folded
  into the PSUM->SBUF evacuation of hh (one tensor_mul with a broadcast tile
  built by a K=1 outer-product matmul; 1/colsum via reciprocal_approx_fast to
  avoid ACT table switches).
  wo ~ 1e-5 means the attention branch contributes ~1e-5 of the output
  (out = x + tiny), so bf16 matmuls (1 cyc/row on PE vs 4 for fp32) are safe.

Scheduling notes:
  - input DMAs ordered x -> emat/ebmat -> wall (x gates the GroupNorm chain)
  - WARM_MM dummy matmuls on a scratch tile warm the PE HAM clock-gate during
    the DMA + GroupNorm head so real matmuls run at 2.4 GHz from the start
  - GroupNorm is emitted phase-wise across the 4 (part, chunk) tiles so the
    DVE FIFO never stalls behind another tile's cross-engine round-trip
  - PSUM->SBUF evacuations alternate between ScalarE and VectorE
  - the frep outer-product matmuls are emitted after the first hh group so
    the PE FIFO doesn't stall waiting for the reciprocal chain
"""

import sys

sys.path.insert(0, "/opt/trn_rl_repo")

import numpy as np
import ml_dtypes

import concourse.bacc as bacc
import concourse.tile as tile
from concourse import mybir
from concourse.bass_utils import run_bass_kernel_spmd

F32 = mybir.dt.float32
BF16 = mybir.dt.bfloat16
AF = mybir.ActivationFunctionType
OP = mybir.AluOpType

B, C, H, W = 8, 256, 32, 32
HW = H * W
G = 32
EPS = 1e-5
NCORES = 8
CK = C // 128      # channel chunks (2)
NK = HW // 512     # free-dim n chunks of 512 (2)
MK = HW // 128     # hw chunks of 128 (8)
GPC = 16           # groups per channel-chunk
WARM_MM = 32       # HAM warm-up matmuls at kernel start


def _widx(proj, kind, ci):
    # proj: 0=q 1=k 2=v 3=o ; kind: 0=re 1=im 2=neg-im ; ci: channel chunk
    return proj * 6 + kind * 2 + ci


def _build_nc(affine_trivial: bool, bias_zero: bool):
    nc = bacc.Bacc("TRN2", target_bir_lowering=False, debug=False)

    x_d = nc.dram_tensor("x", [2, C, HW], F32, kind="ExternalInput")
    w_d = nc.dram_tensor("wall", [128, 24, 256], BF16, kind="ExternalInput")
    e_d = nc.dram_tensor("emat", [128, GPC], F32, kind="ExternalInput")
    eb_d = nc.dram_tensor("ebmat", [GPC, 128], F32, kind="ExternalInput")
    gn_d = None
    if not affine_trivial:
        gn_d = nc.dram_tensor("gnwb", [2, 2, C], F32, kind="ExternalInput")
    bias_d = None
    if not bias_zero:
        bias_d = nc.dram_tensor("bias", [4, 2, C], BF16, kind="ExternalInput")
    out_d = nc.dram_tensor("out", [2, C, HW], F32, kind="ExternalOutput")

    with tile.TileContext(nc) as tc:
        with (
            tc.tile_pool(name="const", bufs=1) as constp,
            tc.tile_pool(name="xp", bufs=1) as xp,
            tc.tile_pool(name="hp", bufs=1) as hp,
            tc.tile_pool(name="qkp", bufs=1) as qkp,
            tc.tile_pool(name="vtp", bufs=1) as vtp,
            tc.tile_pool(name="estp", bufs=1) as estp,
            tc.tile_pool(name="hhp", bufs=1) as hhp,
            tc.tile_pool(name="frp", bufs=1) as frp,
            tc.tile_pool(name="outp", bufs=1) as outp,
            tc.tile_pool(name="smallp", bufs=4) as smallp,
            tc.tile_pool(name="psp", bufs=7, space="PSUM") as psp,
            tc.tile_pool(name="psw", bufs=1, space="PSUM") as psw,
        ):
            # ---- input DMAs (x first: it gates the GroupNorm chain) ----
            xs = [[None] * CK for _ in range(2)]
            for part in range(2):
                for ci in range(CK):
                    xt = xp.tile([128, HW], F32, tag=f"x{part}{ci}")
                    nc.sync.dma_start(xt[:], x_d[part, ci * 128:(ci + 1) * 128, :])
                    xs[part][ci] = xt
            emat = constp.tile([128, GPC], F32, tag="emat")
            nc.sync.dma_start(emat[:], e_d[:])
            ebmat = constp.tile([GPC, 128], F32, tag="ebmat")
            nc.sync.dma_start(ebmat[:], eb_d[:])
            wall = constp.tile([128, 24, 256], BF16, tag="wall")
            nc.sync.dma_start(wall[:], w_d[:])

            eps16 = constp.tile([GPC, 1], F32, tag="eps16")
            nc.vector.memset(eps16[:], EPS)
            onescol = constp.tile([128, 1], BF16, tag="onescol")
            nc.vector.memset(onescol[:], 1.0)
            onesrow = constp.tile([1, 128], F32, tag="onesrow")
            nc.vector.memset(onesrow[:], 1.0)
            warm_sb = constp.tile([128, 512], BF16, tag="warm_sb")
            nc.vector.memset(warm_sb[:], 0.0)
            if not bias_zero:
                onesrow_bf = constp.tile([1, 128], BF16, tag="onesrow_bf")
                nc.vector.memset(onesrow_bf[:], 1.0)
                ones_n = constp.tile([1, 512], BF16, tag="ones_n")
                nc.vector.memset(ones_n[:], 1.0)
                bias_t = [[None, None] for _ in range(4)]
                for pj in range(4):
                    for part in range(2):
                        bt = constp.tile([1, C], BF16, tag=f"bias{pj}{part}")
                        nc.sync.dma_start(
                            bt[:], bias_d[pj, part, :].rearrange("(o c) -> o c", o=1))
                        bias_t[pj][part] = bt
            if not affine_trivial:
                gwb_t = [[None, None] for _ in range(2)]  # [wb][part] -> [128, CK]
                for wb in range(2):
                    for part in range(2):
                        gt = constp.tile([128, CK], F32, tag=f"gn{wb}{part}")
                        nc.sync.dma_start(
                            gt[:], gn_d[wb, part, :].rearrange("(ci p) -> p ci", p=128))
                        gwb_t[wb][part] = gt

            # ---- PE HAM warm-up: dummy matmuls on scratch data ----
            pwarm = psw.tile([128, 512], F32, tag="warm")
            for _ in range(WARM_MM):
                nc.tensor.matmul(pwarm[:], warm_sb[:, 0:128], warm_sb[:],
                                 start=True, stop=True)

            # ---- GroupNorm -> h (bf16), phase-wise across the 4 tiles ----
            tiles4 = [(part, ci) for part in range(2) for ci in range(CK)]
            st6s, mvs, psgs, gcps, gags, gsbs, psbs, scvs = ({} for _ in range(8))
            for part, ci in tiles4:
                xt = xs[part][ci]
                st6 = smallp.tile([128, 2, 6], F32)
                nc.vector.bn_stats(st6[:, 0, :], xt[:, 0:512])
                nc.vector.bn_stats(st6[:, 1, :], xt[:, 512:1024])
                st6s[part, ci] = st6
            for part, ci in tiles4:
                mv = smallp.tile([128, 3], F32)
                nc.vector.bn_aggr(mv[:, 0:2], st6s[part, ci][:])
                nc.vector.tensor_mul(mv[:, 2:3], mv[:, 0:1], mv[:, 0:1])
                mvs[part, ci] = mv
            for part, ci in tiles4:
                # group stats [mean_g, E[var]_g, E[mean^2]_g] (emat = 1/8 indicator)
                psg = psp.tile([GPC, 3], F32, tag="ps")
                nc.tensor.matmul(psg[:], emat[:], mvs[part, ci][:],
                                 start=True, stop=True)
                psgs[part, ci] = psg
            for part, ci in tiles4:
                gcp = smallp.tile([GPC, 3], F32)
                nc.vector.tensor_copy(gcp[:], psgs[part, ci][:])
                gcps[part, ci] = gcp
            for part, ci in tiles4:
                gcp = gcps[part, ci]
                gag = smallp.tile([GPC, 4], F32)
                nc.vector.tensor_add(gag[:, 0:1], gcp[:, 1:2], gcp[:, 2:3])
                nc.vector.tensor_mul(gag[:, 1:2], gcp[:, 0:1], gcp[:, 0:1])
                nc.vector.tensor_sub(gag[:, 2:3], gag[:, 0:1], gag[:, 1:2])
                nc.scalar.activation(gag[:, 3:4], gag[:, 2:3], AF.Sqrt, bias=eps16[:])
                gags[part, ci] = gag
            for part, ci in tiles4:
                gsb = smallp.tile([GPC, 2], F32)
                nc.vector.reciprocal(gsb[:, 1:2], gags[part, ci][:, 3:4])
                nc.vector.tensor_copy(gsb[:, 0:1], gcps[part, ci][:, 0:1])
                gsbs[part, ci] = gsb
            for part, ci in tiles4:
                psb = psp.tile([128, 2], F32, tag="ps")
                nc.tensor.matmul(psb[:], ebmat[:], gsbs[part, ci][:],
                                 start=True, stop=True)
                psbs[part, ci] = psb
            hs = [[None] * CK for _ in range(2)]
            for part, ci in tiles4:
                scv = smallp.tile([128, 2], F32)
                nc.vector.tensor_copy(scv[:], psbs[part, ci][:])
                ht = hp.tile([128, HW], BF16, tag=f"h{part}{ci}")
                if affine_trivial:
                    nc.vector.tensor_scalar(
                        out=ht[:], in0=xs[part][ci][:],
                        scalar1=scv[:, 0:1], scalar2=scv[:, 1:2],
                        op0=OP.subtract, op1=OP.mult)
                else:
                    ab = smallp.tile([128, 2], F32)
                    # A = rstd * gn_w ; B = gn_b - mean * A
                    nc.vector.tensor_mul(
                        ab[:, 0:1], scv[:, 1:2], gwb_t[0][part][:, ci:ci + 1])
                    nc.vector.tensor_mul(ab[:, 1:2], scv[:, 0:1], ab[:, 0:1])
                    nc.vector.tensor_sub(
                        ab[:, 1:2], gwb_t[1][part][:, ci:ci + 1], ab[:, 1:2])
                    nc.vector.tensor_scalar(
                        out=ht[:], in0=xs[part][ci][:],
                        scalar1=ab[:, 0:1], scalar2=ab[:, 1:2],
                        op0=OP.mult, op1=OP.add)
                hs[part][ci] = ht

            # ---- q, k projections (out layout [o, n]) ----
            # q_re = Wqr h_re - Wqi h_im ; q_im = Wqr h_im + Wqi h_re
            # second warm-up batch: keeps HAM warm across the GroupNorm lull
            for _ in range(12):
                nc.tensor.matmul(pwarm[:], warm_sb[:, 0:128], warm_sb[:],
                                 start=True, stop=True)

            qk = [[[None] * CK for _ in range(2)] for _ in range(2)]  # [pj][part][mo]
            for pj in range(2):  # 0=q 1=k
                for part in range(2):
                    terms = [(0, hs[0]), (2, hs[1])] if part == 0 else \
                            [(0, hs[1]), (1, hs[0])]
                    for mo in range(CK):
                        qt = qkp.tile([128, HW], BF16, tag=f"qk{pj}{part}{mo}",
                                      name=f"qk{pj}{part}{mo}")
                        qk[pj][part][mo] = qt
                        for nn in range(NK):
                            ps = psp.tile([128, 512], F32, tag="ps")
                            nmm = 4 if bias_zero else 5
                            mm = 0
                            for kind, hsrc in terms:
                                for ci in range(CK):
                                    nc.tensor.matmul(
                                        ps[:],
                                        wall[:, _widx(pj, kind, ci),
                                             mo * 128:(mo + 1) * 128],
                                        hsrc[ci][:, nn * 512:(nn + 1) * 512],
                                        start=(mm == 0), stop=(mm == nmm - 1))
                                    mm += 1
                            if not bias_zero:
                                nc.tensor.matmul(
                                    ps[:],
                                    bias_t[pj][part][:, mo * 128:(mo + 1) * 128],
                                    ones_n[:], start=False, stop=True)
                            nc.scalar.copy(qt[:, nn * 512:(nn + 1) * 512], ps[:])

            # ---- v^T (layout [m, o]) ----
            vt = [None, None]
            for part in range(2):
                vt[part] = vtp.tile([128, MK, 256], BF16, tag=f"vt{part}",
                                    name=f"vt{part}")
            for part in range(2):
                terms = [(hs[0], 0), (hs[1], 2)] if part == 0 else \
                        [(hs[1], 0), (hs[0], 1)]
                for mk in range(MK):
                    ps = psp.tile([128, 256], F32, tag="ps")
                    nmm = 4 if bias_zero else 5
                    mm = 0
                    for hsrc, kind in terms:
                        for ci in range(CK):
                            nc.tensor.matmul(
                                ps[:],
                                hsrc[ci][:, mk * 128:(mk + 1) * 128],
                                wall[:, _widx(2, kind, ci), :],
                                start=(mm == 0), stop=(mm == nmm - 1))
                            mm += 1
                    if not bias_zero:
                        nc.tensor.matmul(ps[:], onesrow_bf[:], bias_t[2][part][:],
                                         start=False, stop=True)
                    nc.scalar.copy(vt[part][:, mk, :], ps[:])

            # ---- St = k^T q (1/16 folded into wq) -> exp (bf16) ----
            est = [None] * MK
            for mk in range(MK):
                est[mk] = estp.tile([128, HW], BF16, tag=f"est{mk}", name=f"est{mk}")
            for mk in range(MK):
                for nn in range(NK):
                    ps = psp.tile([128, 512], F32, tag="ps")
                    mm = 0
                    for part in range(2):
                        for ci in range(CK):
                            nc.tensor.matmul(
                                ps[:],
                                qk[1][part][ci][:, mk * 128:(mk + 1) * 128],
                                qk[0][part][ci][:, nn * 512:(nn + 1) * 512],
                                start=(mm == 0), stop=(mm == 3))
                            mm += 1
                    nc.scalar.activation(
                        est[mk][:, nn * 512:(nn + 1) * 512], ps[:], AF.Exp)

            # ---- column sums -> 1/colsum (DVE approx reciprocal) ----
            ivcs = [None] * NK
            for nn in range(NK):
                psc = psp.tile([1, 512], F32, tag="ps")
                for mk in range(MK):
                    nc.tensor.matmul(
                        psc[:], onescol[:], est[mk][:, nn * 512:(nn + 1) * 512],
                        start=(mk == 0), stop=(mk == MK - 1))
                ivc = smallp.tile([1, 512], F32)
                nc.vector.reciprocal_approx_fast(out=ivc[:], in_=psc[:])
                ivcs[nn] = ivc

            # ---- hh = v^T.T @ expSt, normalized during evacuation ----
            # frep (ones outer ivc) is emitted after the first hh group so the
            # PE FIFO never waits on the reciprocal chain.
            hh = [[None] * CK for _ in range(2)]
            frep = [None] * NK
            for part in range(2):
                for co in range(CK):
                    hh[part][co] = hhp.tile([128, HW], BF16, tag=f"hh{part}{co}",
                                            name=f"hh{part}{co}")
            # nn-major: all hh groups of one nn run back-to-back on PE, their
            # evacuations overlap the next nn's matmuls, then z streams after.
            for nn in range(NK):
                for gi, (part, co) in enumerate(
                        [(p, c) for p in range(2) for c in range(CK)]):
                    ps = psp.tile([128, 512], F32, tag="ps")
                    for mk in range(MK):
                        nc.tensor.matmul(
                            ps[:],
                            vt[part][:, mk, co * 128:(co + 1) * 128],
                            est[mk][:, nn * 512:(nn + 1) * 512],
                            start=(mk == 0), stop=(mk == MK - 1))
                    if nn == 0 and gi == 0:
                        for fn in range(NK):
                            psf = psp.tile([128, 512], F32, tag="ps")
                            nc.tensor.matmul(psf[:], onesrow[:], ivcs[fn][:],
                                             start=True, stop=True)
                            ft = frp.tile([128, 512], F32, tag=f"frep{fn}",
                                          name=f"frep{fn}")
                            nc.vector.tensor_copy(ft[:], psf[:])
                            frep[fn] = ft
                    nc.vector.tensor_mul(
                        hh[part][co][:, nn * 512:(nn + 1) * 512], ps[:], frep[nn][:])

            # ---- z = Wo hh (complex), out = x + z ----
            outt = [[None] * CK for _ in range(2)]
            for part in range(2):
                for mo in range(CK):
                    outt[part][mo] = outp.tile(
                        [128, HW], F32, tag=f"out{part}{mo}", name=f"out{part}{mo}")
            for nn in range(NK):
                for part in range(2):
                    terms = [(0, hh[0]), (2, hh[1])] if part == 0 else \
                            [(0, hh[1]), (1, hh[0])]
                    for mo in range(CK):
                        ps = psp.tile([128, 512], F32, tag="ps")
                        nmm = 4 if bias_zero else 5
                        mm = 0
                        for kind, hsrc in terms:
                            for ci in range(CK):
                                nc.tensor.matmul(
                                    ps[:],
                                    wall[:, _widx(3, kind, ci),
                                         mo * 128:(mo + 1) * 128],
                                    hsrc[ci][:, nn * 512:(nn + 1) * 512],
                                    start=(mm == 0), stop=(mm == nmm - 1))
                                mm += 1
                        if not bias_zero:
                            nc.tensor.matmul(
                                ps[:],
                                bias_t[3][part][:, mo * 128:(mo + 1) * 128],
                                ones_n[:], start=False, stop=True)
                        nc.vector.tensor_add(
                            outt[part][mo][:, nn * 512:(nn + 1) * 512], ps[:],
                            xs[part][mo][:, nn * 512:(nn + 1) * 512])
                        nc.sync.dma_start(
                            out_d[part, mo * 128:(mo + 1) * 128,
                                  nn * 512:(nn + 1) * 512],
                            outt[part][mo][:, nn * 512:(nn + 1) * 512])

    nc.compile()
    return nc


_NC_CACHE = {}


def _get_nc(affine_trivial, bias_zero):
    key = (affine_trivial, bias_zero)
    if key not in _NC_CACHE:
        _NC_CACHE[key] = _build_nc(affine_trivial, bias_zero)
    return _NC_CACHE[key]


def _host_inputs(x2, gn_w, gn_b, wq, bq, wk, bk, wv, bv, wo, bo):
    bf = ml_dtypes.bfloat16
    sq = 1.0 / np.sqrt(np.float32(C))
    blocks = []
    for w, s in ((wq, sq), (wk, 1.0), (wv, 1.0), (wo, 1.0)):
        wr = np.asarray(w[0], np.float32).T * s
        wi = np.asarray(w[1], np.float32).T * s
        for mat in (wr, wi, -wi):
            for ci in range(CK):
                blocks.append(mat[ci * 128:(ci + 1) * 128, :])
    wall = np.ascontiguousarray(
        np.stack(blocks).transpose(1, 0, 2)).astype(bf)  # [128, 24, 256]

    emat = np.zeros((128, GPC), np.float32)
    ebmat = np.zeros((GPC, 128), np.float32)
    for c in range(128):
        emat[c, c // 8] = 0.125
        ebmat[c // 8, c] = 1.0

    gn_w = np.asarray(gn_w, np.float32)
    gn_b = np.asarray(gn_b, np.float32)
    affine_trivial = bool(np.all(gn_w == 1.0) and np.all(gn_b == 0.0))
    biases = np.stack([np.asarray(b, np.float32) for b in (bq, bk, bv, bo)])
    bias_zero = bool(np.all(biases == 0.0))

    shared = {"wall": wall, "emat": emat, "ebmat": ebmat}
    if not affine_trivial:
        shared["gnwb"] = np.ascontiguousarray(np.stack([gn_w, gn_b]))
    if not bias_zero:
        shared["bias"] = np.ascontiguousarray(biases).astype(bf)

    x2 = np.asarray(x2, np.float32)
    in_maps = []
    for b in range(B):
        m = dict(shared)
        m["x"] = np.ascontiguousarray(x2[:, b].reshape(2, C, HW))
        in_maps.append(m)
    return in_maps, affine_trivial, bias_zero


def kernel(x2, gn_w, gn_b, wq, bq, wk, bk, wv, bv, wo, bo, _profile_dir=None):
    in_maps, affine_trivial, bias_zero = _host_inputs(
        x2, gn_w, gn_b, wq, bq, wk, bk, wv, bv, wo, bo)
    nc = _get_nc(affine_trivial, bias_zero)

    if _profile_dir is not None:
        import ctypes, os
        import jax
        jax.devices()
        lib = ctypes.CDLL("/opt/axon/libaxon_pjrt.so")
        lib.axon_start_nrt_profile.argtypes = [
            ctypes.POINTER(ctypes.c_int64), ctypes.c_size_t]
        lib.axon_start_nrt_profile.restype = ctypes.c_int64
        lib.axon_stop_nrt_profile.argtypes = [ctypes.c_char_p]
        lib.axon_stop_nrt_profile.restype = ctypes.c_int64
        os.makedirs(_profile_dir, exist_ok=True)
        ids = (ctypes.c_int64 * NCORES)(*range(NCORES))
        rc = lib.axon_start_nrt_profile(ids, NCORES)
        if rc != 0:
            raise RuntimeError(f"axon_start_nrt_profile rc={rc}")
        try:
            res = run_bass_kernel_spmd(nc, in_maps, list(range(NCORES)))
        finally:
            n = lib.axon_stop_nrt_profile(_profile_dir.encode())
            print(f"profile: {n} file(s) written to {_profile_dir}")
    else:
        res = run_bass_kernel_spmd(nc, in_maps, list(range(NCORES)))

    out = np.stack(
        [np.asarray(res.results[b]["out"], np.float32) for b in range(B)], axis=1)
    return np.ascontiguousarray(out.reshape(2, B, C, H, W))
